# revision 42
# baseline (speedup 1.0000x reference)
"""HGT GNN kernel for 8 Trainium2 NeuronCores — v2.

Device does all heavy dense matmuls in bf16 via three cached Bass/Tile
programs, each covering all three node types in one call (per-row-block
weight selection, exact per-core row counts, no padding):

  pin   : x[Ni,128]    @ W_in[i]                      -> 256 cols out
  fused : h[Ni,256]    @ [Wq | Wk@Wkrel_e | Wv@Wvrel_e] -> q/kr/vr in one shot
          (the K/V relation projections are folded into the KQV weights,
          removing the separate k/v matmuls and 8 relation matmuls/layer)
  wout  : gelu(aggr)   @ W_out[l,i]                   -> 256 cols out

JumpingKnowledge + SAG pooling are algebraically folded to the host side:
gate scores use W_jk@W_gate, and pooled = segsum(w*cat) @ W_jk, so the
[170000,1024]@[1024,256] JK matmul disappears entirely.

Irregular glue (edge gather / segment softmax / scatter) and the tiny
BatchNorm head run on host in fp32, with edges presorted by destination.
"""

import contextlib
import ctypes
import sys
import types

import numpy as np
import ml_dtypes

import concourse.bass as bass
import concourse.mybir as mybir
import concourse.tile as tile
from concourse.bass_utils import run_bass_kernel_spmd
from concourse.vector_clock import ScopedClock

BF16 = ml_dtypes.bfloat16


# ------------------------------------------------------- ntff profile shim
def _install_ntff_shim():
    """This image's antenv lacks axon_hooks; recreate the NTFF profile hook
    via the libaxon_pjrt.so C ABI so trace=True yields exec_time_ns."""
    try:
        from antenv.axon_hooks import get_axon_ntff_profile_hook  # noqa: F401
        return
    except ImportError:
        pass

    so_path = "/opt/axon/libaxon_pjrt.so"
    try:
        lib = ctypes.CDLL(so_path)
    except OSError:
        return
    if not hasattr(lib, "axon_start_nrt_profile"):
        return
    lib.axon_start_nrt_profile.argtypes = [ctypes.POINTER(ctypes.c_int64),
                                           ctypes.c_size_t]
    lib.axon_start_nrt_profile.restype = ctypes.c_int64
    lib.axon_stop_nrt_profile.argtypes = [ctypes.c_char_p]
    lib.axon_stop_nrt_profile.restype = ctypes.c_int64

    @contextlib.contextmanager
    def _hook(output_dir, device_ids):
        import jax
        jax.devices()
        if device_ids:
            ids = (ctypes.c_int64 * len(device_ids))(*device_ids)
            rc = lib.axon_start_nrt_profile(ids, len(device_ids))
        else:
            rc = lib.axon_start_nrt_profile(None, 0)
        if rc != 0:
            raise RuntimeError(f"axon_start_nrt_profile rc={rc}")
        try:
            yield
        finally:
            n = lib.axon_stop_nrt_profile(str(output_dir).encode())
            if n <= 0:
                print(f"ntff profile capture wrote {n} files", file=sys.stderr)

    mod = types.ModuleType("antenv.axon_hooks")
    mod.get_axon_ntff_profile_hook = lambda: _hook
    mod.set_axon_ntff_profile_hook = lambda h: None
    sys.modules["antenv.axon_hooks"] = mod
    import antenv
    antenv.axon_hooks = mod

    import concourse.bass_utils as bu
    bu.upload_artifacts = lambda tmpdir: tmpdir


_install_ntff_shim()

# model dims (hardcoded per contract)
H, DH, F, L, B = 4, 64, 256, 4, 64
NS = [80000, 60000, 30000]
ET = [(0, 1), (1, 0), (0, 2), (2, 0)]
NE = [320000, 320000, 160000, 160000]
CIN = 128

N_CORES = 8
PC = [n // N_CORES for n in NS]          # 10000, 7500, 3750 rows/core
RTOT = sum(PC)                           # 21250
COFF = [0, PC[0], PC[0] + PC[1]]         # per-type col offsets in device layout
FB = 512                                 # free-dim block = one PSUM bank exactly


# ---------------------------------------------------------------- tile drain fix
def _install_tilefix():
    """This container's walrus rejects >1 sync wait on TPB_CTRL-class
    instructions; spread the Tile tail-drain waits across SP nops."""

    def _drain_and_barrier_split(self, tick_clock, wait_clock):
        """Minimal teardown: drain the two DMA-issuing engines (guarantees
        all output DMAs complete), one engine barrier, then clear sems from
        gpsimd without a second barrier or per-tile-clock waits."""
        nc = self.nc
        nc.sync.drain()
        nc.gpsimd.drain()
        nc.all_engine_barrier()
        assert self.sems is not None
        popped = nc._tile_sem_poison_stack.pop()
        assert popped is self._sem_poison
        sems = list(self.sems.allocated().values())
        sem_nums = [s.num if hasattr(s, "num") else s for s in sems]
        from concourse.bass import compact_to_ranges
        for sem_range in compact_to_ranges(sem_nums):
            nc.gpsimd.dma_reset(sem_range)
            nc.gpsimd.sem_clear(sem_range)
        nc._state.prepend_free_semaphores(sem_nums)

    tile.TileContext._drain_and_barrier = _drain_and_barrier_split


_install_tilefix()


def _split_multiwaits(nc):
    """Walrus here allows only one sync wait per instruction: move extra
    waits onto same-engine nops placed immediately before the instruction."""
    for f in nc.m.functions:
        for bb in f.blocks:
            insts = list(bb.instructions)
            out = []
            for inst in insts:
                si = getattr(inst, "sync_info", None)
                if si and si.on_wait and len(si.on_wait) > 1:
                    extra, keep = si.on_wait[:-1], si.on_wait[-1:]
                    si.on_wait = keep
                    for w in extra:
                        nop = nc.engines[inst.engine].nop(nofuse=True)
                        cur = nc.cur_bb.bb.instructions
                        assert cur[-1] is nop.ins
                        cur.pop()
                        nop.ins.sync_info = type(si)(on_wait=[w], on_update=[])
                        out.append(nop.ins)
                out.append(inst)
            bb.instructions[:] = out


# ---------------------------------------------------------------- device matmul
_PROGS = {}
_CALL_COUNTS = {}


GW = 2048  # column group width: 4 PSUM banks per (group, mc); one in/out DMA per group
OUT_ENG = "gpsimd"  # which engine issues output DMAs: sync | scalar | gpsimd


def _groups():
    """(type, group_col0, group_width) covering each type's per-core cols."""
    out = []
    for t in range(3):
        n, c0 = PC[t], COFF[t]
        g = 0
        while g < n:
            w = min(GW, n - g)
            out.append((t, c0 + g, w))
            g += w
    return out


WSCALE = 16.0  # fp8 weight pre-scale for DoubleRow k-columns


def _build_multi(K, Ms, M8, in8=False, drk=False):
    """One SPMD program: per-type matmuls over the concatenated per-core
    rows. xt [K, RTOT] bf16 (feature-major), w [K, sum(Ms)] bf16.
    The first M8 output rows of each type are written to yt8 (fp8e4),
    the rest to yt (bf16). DMA is coalesced at GW-column granularity
    (MB-scale transfers); each (group, mc) accumulates into a 2-bank PSUM
    tile drained by a single wide PSUM->SBUF cast, alternating DVE/ACT.
    ~7us of scratch warmup matmuls run during the input DMA ramp so the
    PE HAM clock-gate is at 8/8 before the first real matmul."""
    dt = mybir.dt.bfloat16
    dt8 = mybir.dt.float8e4
    KC = K // 128
    Mtot, Mmax = sum(Ms), max(Ms)
    MCmax = Mmax // 128
    MC8 = M8 // 128                      # leading fp8 out blocks per type
    MC16max = MCmax - MC8
    woff = [0, Ms[0], Ms[0] + Ms[1]]
    dtin = dt8 if in8 else dt
    # drk: mc blocks [MC8//2, MC8) of each type (the fused "k" columns) are
    # computed as a single DoubleRow fp8 matmul over both K halves, using a
    # device-side fp8 cast of x and pre-scaled fp8 weights from `w8`.
    DRLO = MC8 // 2 if drk else None
    nc = bass.Bass("TRN2", target_bir_lowering=False, debug=False,
                   num_devices=N_CORES)
    # group-blocked contiguous layouts: element (p, kc|mc, c) of column-group
    # g lives at flat offset g0*K + (p*KC + kc)*gwid + c, so every DMA is one
    # contiguous DRAM segment per partition (max-size descriptors).
    xt = nc.dram_tensor("xt", [RTOT * K], dtin, kind="ExternalInput")
    w = nc.dram_tensor("w", [K, Mtot], dt, kind="ExternalInput")
    if drk:
        MDR = 128 * (MC8 - DRLO) * len(Ms)    # DR cols across all types
        w8 = nc.dram_tensor("w8", [K, MDR], dt8, kind="ExternalInput")
    if MC8 > 0:
        yt8 = nc.dram_tensor("yt8", [RTOT * M8], dt8, kind="ExternalOutput")
    if MC16max > 0:
        M16u = Mmax - M8
        yt = nc.dram_tensor("yt", [RTOT * M16u], dt, kind="ExternalOutput")
    grs = _groups()

    def xblk(g0, gwid):
        return xt[g0 * K:(g0 + gwid) * K].rearrange(
            "(p kc c) -> p kc c", p=128, kc=KC)

    def y8blk(g0, gwid):
        return yt8[g0 * M8:(g0 + gwid) * M8].rearrange(
            "(p mc c) -> p mc c", p=128, mc=MC8)

    def y16blk(g0, gwid):
        return yt[g0 * M16u:(g0 + gwid) * M16u].rearrange(
            "(p mc c) -> p mc c", p=128, mc=MC16max)
    # og tile is MCmax*GW*2 bytes/partition; budget ~208KB/partition
    op_bufs = 4 if MCmax >= 10 else (6 if MCmax >= 6 else 8)
    with tile.TileContext(nc) as tc:
        with (
            tc.tile_pool(name="wp", bufs=1) as wp,
            tc.tile_pool(name="xp", bufs=6) as xp,
            tc.tile_pool(name="op", bufs=op_bufs) as op,
            tc.tile_pool(name="ps", bufs=4, space="PSUM") as ps,
        ):
            # --- PE warmup: scratch matmuls with no DMA dependencies ---
            # (shares the "ph" PSUM ring so it costs no extra PSUM space)
            # ~12 N=256 matmuls span one HAM window at the cold rate, so the
            # clock-gate is at 8/8 just as the first real inputs land.
            wu = wp.tile([128, 384], dt)
            nc.vector.memset(wu[:, :], 0.0)
            wups = ps.tile([128, 2 * FB], mybir.dt.float32, space="PSUM",
                           name="ph")
            for _ in range(13):
                nc.tensor.matmul(out=wups[:, :256], lhsT=wu[:, :128],
                                 rhs=wu[:, 128:384],
                                 start=True, stop=True)
            wt = wp.tile([128, KC * Mtot], dt)
            M0 = Ms[0]
            wdmas = []
            for kc in range(KC):   # first-needed weight cols (type 0)
                wdmas.append((wt[:, kc * Mtot:kc * Mtot + M0],
                              w[kc * 128:(kc + 1) * 128, :M0]))
            if drk:
                assert KC == 2, "DoubleRow path assumes K=256"
                wt8 = wp.tile([128, KC, MDR], dt8)
                for kc in range(KC):
                    wdmas.append((wt8[:, kc, :],
                                  w8[kc * 128:(kc + 1) * 128, :]))
            for kc in range(KC):   # remaining weight cols
                if Mtot > M0:
                    wdmas.append((wt[:, kc * Mtot + M0:(kc + 1) * Mtot],
                                  w[kc * 128:(kc + 1) * 128, M0:]))
            HW2 = 2 * FB          # half-group: one 2-bank PSUM tile
            xgs = {}
            for gi, (t, g0, gwid) in enumerate(grs):
                MC = Ms[t] // 128
                MC16 = MC - MC8
                xg = xp.tile([128, KC, GW], dtin, name="xg")
                xgs[gi] = xg
                xb = xblk(g0, gwid)
                if gi == 0:
                    # first matmul's weights first, then a fine-grained
                    # first x chunk so matmul 0 starts early
                    for o, i_ in wdmas[:KC]:
                        nc.sync.dma_start(out=o, in_=i_)
                    h1 = min(FB, gwid)
                    nc.sync.dma_start(out=xg[:, :, :h1], in_=xb[:, :, :h1])
                    for o, i_ in wdmas[KC:]:
                        nc.sync.dma_start(out=o, in_=i_)
                    if gwid > h1:
                        nc.sync.dma_start(out=xg[:, :, h1:gwid],
                                          in_=xb[:, :, h1:gwid])
                else:
                    nc.sync.dma_start(out=xg[:, :, :gwid],
                                      in_=xb[:, :, :gwid])
                if drk:  # fp8 copy of x for the DoubleRow k-matmuls
                    xg8 = xp.tile([128, KC, GW], dt8, name="xg8", bufs=4)
                    nc.gpsimd.tensor_copy(out=xg8[:, :, :gwid],
                                          in_=xg[:, :, :gwid])
                og8 = (op.tile([128, MC8, GW], dt8, name="og8")
                       if MC8 else None)
                og16 = (op.tile([128, MC16, GW], dt, name="og16")
                        if MC16 else None)
                last = gi == len(grs) - 1
                for mc in range(MC):
                    if mc < MC8:
                        og, ogmc = og8, mc
                    else:
                        og, ogmc = og16, mc - MC8
                    isdr = drk and DRLO <= mc < MC8
                    for hb, h0 in enumerate(range(0, gwid, HW2)):
                        hw = min(HW2, gwid - h0)
                        ph = ps.tile([128, HW2], mybir.dt.float32,
                                     space="PSUM")
                        if isdr:   # one DoubleRow matmul covers both K halves
                            mdr = (t * (MC8 - DRLO) + (mc - DRLO)) * 128
                            for boff in range(h0, h0 + hw, FB):
                                fb = min(FB, h0 + hw - boff)
                                nc.tensor.matmul(
                                    out=ph[:, boff - h0:boff - h0 + fb],
                                    lhsT=wt8[:, :, mdr:mdr + 128],
                                    rhs=xg8[:, :, boff:boff + fb],
                                    start=True, stop=True,
                                    perf_mode=mybir.MatmulPerfMode.DoubleRow)
                        else:
                            for kc in range(KC):   # blocks inner: LDW shared
                                wcol = kc * Mtot + woff[t] + mc * 128
                                for boff in range(h0, h0 + hw, FB):
                                    fb = min(FB, h0 + hw - boff)
                                    nc.tensor.matmul(
                                        out=ph[:, boff - h0:boff - h0 + fb],
                                        lhsT=wt[:, wcol:wcol + 128],
                                        rhs=xg[:, kc, boff:boff + fb],
                                        start=(kc == 0), stop=(kc == KC - 1))
                        # the two halves of each mc drain on different engines
                        # (DoubleRow columns also undo the weight pre-scale)
                        if hb % 2 == 0:
                            if isdr:
                                nc.vector.tensor_scalar_mul(
                                    out=og[:, ogmc, h0:h0 + hw],
                                    in0=ph[:, :hw], scalar1=1.0 / WSCALE)
                            else:
                                nc.vector.tensor_copy(
                                    out=og[:, ogmc, h0:h0 + hw],
                                    in_=ph[:, :hw])
                        else:
                            if isdr:
                                nc.scalar.mul(out=og[:, ogmc, h0:h0 + hw],
                                              in_=ph[:, :hw],
                                              mul=1.0 / WSCALE)
                            else:
                                nc.scalar.copy(out=og[:, ogmc, h0:h0 + hw],
                                               in_=ph[:, :hw])
                    if last:  # drain the final group per-mc: shorter tail
                        # alternate issuing engines so descriptor generation
                        # for the final DMAs overlaps
                        eng = ("gpsimd", "sync")[mc % 2]
                        if mc < MC8:
                            getattr(nc, eng).dma_start(
                                out=y8blk(g0, gwid)[:, mc:mc + 1, :gwid],
                                in_=og8[:, mc:mc + 1, :gwid])
                        else:
                            getattr(nc, eng).dma_start(
                                out=y16blk(g0, gwid)[:, mc - MC8:mc - MC8 + 1,
                                                     :gwid],
                                in_=og16[:, mc - MC8:mc - MC8 + 1, :gwid])
                if not last:
                    if MC8:
                        getattr(nc, OUT_ENG).dma_start(
                            out=y8blk(g0, gwid)[:, :MC8, :gwid],
                            in_=og8[:, :MC8, :gwid])
                    if MC16:
                        getattr(nc, OUT_ENG).dma_start(
                            out=y16blk(g0, gwid)[:, :MC16, :gwid],
                            in_=og16[:, :MC16, :gwid])
    _split_multiwaits(nc)
    return nc


def _make_runner(nc, Mmax, M8, in8=False, drk=False):
    """Persistent jitted SPMD executor for one program (built once;
    per-call dispatch is then cheap)."""
    import jax
    import jax.numpy as jnp
    from jax.experimental.shard_map import shard_map
    from jax.sharding import Mesh, PartitionSpec
    from concourse.bass2jax import (_bass_exec_p, partition_id_tensor,
                                    install_neuronx_cc_hook)

    install_neuronx_cc_hook()
    F8 = ml_dtypes.float8_e4m3fn
    M16 = Mmax - M8
    out_names, out_avals, zdts, zrows = [], [], [], []
    if M8:
        out_names.append("yt8")
        out_avals.append(jax.core.ShapedArray((RTOT * M8,), F8))
        zdts.append(F8)
        zrows.append(M8)
    if M16:
        out_names.append("yt")
        out_avals.append(jax.core.ShapedArray((RTOT * M16,), BF16))
        zdts.append(BF16)
        zrows.append(M16)
    pname = nc.partition_id_tensor.name if nc.partition_id_tensor else None
    wnames = ["w", "w8"] if drk else ["w"]
    in_names = ["xt"] + wnames + out_names + ([pname] if pname else [])
    nin = 1 + len(wnames)

    def _body(xt, *rest):
        operands = [xt, *rest]
        if pname is not None:
            operands.append(partition_id_tensor())
        outs = _bass_exec_p.bind(
            *operands, out_avals=tuple(out_avals), in_names=tuple(in_names),
            out_names=tuple(out_names), lowering_input_output_aliases=(),
            sim_require_finite=False, sim_require_nnan=False, nc=nc)
        return tuple(outs)

    devices = jax.devices()[:N_CORES]
    mesh = Mesh(np.asarray(devices), ("core",))
    nouts = len(out_names)
    sharded = jax.jit(
        shard_map(_body, mesh=mesh,
                  in_specs=(PartitionSpec("core"),) * (nin + nouts),
                  out_specs=(PartitionSpec("core"),) * nouts,
                  check_rep=False),
        keep_unused=True)
    yzs = [jax.device_put(
        np.zeros(N_CORES * RTOT * r, zdt),
        jax.sharding.NamedSharding(mesh, PartitionSpec("core")))
        for r, zdt in zip(zrows, zdts)]

    def run(xt_all, wcat, w8cat=None):
        # xt_all [N_CORES * RTOT*K] packed; wcat [K, Mtot] bf16 (replicated)
        def rep(a):
            return np.ascontiguousarray(
                np.broadcast_to(a, (N_CORES,) + a.shape)
            ).reshape((N_CORES * a.shape[0],) + a.shape[1:])
        ws = [rep(wcat)] + ([rep(w8cat)] if drk else [])
        outs = sharded(xt_all, *ws, *yzs)
        r8 = np.asarray(outs[0]) if M8 else None
        r16 = np.asarray(outs[-1]) if M16 else None
        return r8, r16

    return run


def _get_prog(K, Ms, M8=0, in8=False, drk=False):
    key = (K,) + tuple(Ms) + (M8, in8, drk)
    if key not in _PROGS:
        nc = _build_multi(K, Ms, M8, in8, drk)
        _PROGS[key] = (nc, _make_runner(nc, max(Ms), M8, in8, drk),
                       Ms, M8, in8, drk)
    return _PROGS[key]


def _dev_call(K, Ms, xs_by_type, wcat, M8=0, in8=False, drk=False):
    """xs_by_type: list of 3 host arrays [N_i, K] f32.  wcat [K, sum(Ms)] f32.
    Returns list of 3 arrays [N_i, Ms[i]] f32; the first M8 output cols of
    each type round-trip through fp8e4m3; in8 ships x as fp8e4m3; drk runs
    the k-columns (second quarter of M8) as DoubleRow fp8 matmuls."""
    key = (K,) + tuple(Ms) + (M8, in8, drk)
    _, run, _, _, _, _ = _get_prog(K, Ms, M8, in8, drk)
    _CALL_COUNTS[key] = _CALL_COUNTS.get(key, 0) + 1
    Mmax = max(Ms)
    M16 = Mmax - M8
    KC = K // 128
    MC8 = M8 // 128
    F8 = ml_dtypes.float8_e4m3fn
    dtin = F8 if in8 else BF16
    grs = _groups()
    # pack: per core, per column-group, block layout (p, kc, c) contiguous
    xt_all = np.empty((N_CORES, RTOT * K), dtin)
    for c in range(N_CORES):
        for t in range(3):
            pc = PC[t]
            xkm = np.ascontiguousarray(
                xs_by_type[t][c * pc:(c + 1) * pc].T).astype(dtin)  # [K, pc]
            xkm = xkm.reshape(KC, 128, pc).transpose(1, 0, 2)       # [p,kc,pc]
            for (gt, g0, gwid) in grs:
                if gt != t:
                    continue
                lg = g0 - COFF[t]
                xt_all[c, g0 * K:(g0 + gwid) * K] = \
                    xkm[:, :, lg:lg + gwid].ravel()
    if drk:
        F2 = M8 // 2                       # k-cols are [F2, 2*F2) per type
        woffs = np.cumsum([0] + list(Ms))
        w8cat = np.concatenate(
            [wcat[:, woffs[t] + F2:woffs[t] + 2 * F2]
             for t in range(len(Ms))], axis=1)
        w8cat = (w8cat * WSCALE).astype(F8)
        y8all, y16all = run(xt_all.reshape(-1), wcat.astype(BF16), w8cat)
    else:
        y8all, y16all = run(xt_all.reshape(-1), wcat.astype(BF16))
    outs = []
    for t in range(3):
        pc, Mt = PC[t], Ms[t]
        parts = []
        for c in range(N_CORES):
            arr = np.empty((pc, Mt), np.float32)
            for (gt, g0, gwid) in grs:
                if gt != t:
                    continue
                lg = g0 - COFF[t]
                if M8:
                    blk = y8all[c * RTOT * M8 + g0 * M8:
                                c * RTOT * M8 + (g0 + gwid) * M8]
                    blk = blk.reshape(128, MC8, gwid).transpose(2, 1, 0)
                    arr[lg:lg + gwid, :M8] = \
                        blk.reshape(gwid, M8).astype(np.float32)
                if Mt - M8 > 0:
                    blk = y16all[c * RTOT * M16 + g0 * M16:
                                 c * RTOT * M16 + (g0 + gwid) * M16]
                    blk = blk.reshape(128, M16 // 128, gwid).transpose(2, 1, 0)
                    arr[lg:lg + gwid, M8:M8 + M16] = \
                        blk.reshape(gwid, M16).astype(np.float32)
            parts.append(arr)
        outs.append(np.concatenate(parts, axis=0))
    return outs


def _timed_mm_ns():
    """Three traced runs per cached program (min, to reject power-state
    outliers); returns sum(count * exec_ns)."""
    total = 0
    for key, (nc, _run, Ms, M8, in8, drk) in _PROGS.items():
        K = key[0]
        dtin = ml_dtypes.float8_e4m3fn if in8 else BF16
        in_maps = [{"xt": np.zeros(RTOT * K, dtin),
                    "w": np.zeros((K, sum(Ms)), BF16)}
                   for _ in range(N_CORES)]
        if drk:
            F8z = np.zeros((K, (M8 // 2) * len(Ms)),
                           ml_dtypes.float8_e4m3fn)
            for m in in_maps:
                m["w8"] = F8z
        times = []
        for _ in range(3):
            r = run_bass_kernel_spmd(nc, in_maps, list(range(N_CORES)),
                                     trace=True)
            if r.exec_time_ns:
                times.append(r.exec_time_ns)
        if times:
            total += min(times) * _CALL_COUNTS.get(key, 0)
    return total


# ---------------------------------------------------------------- host helpers
def _gelu(x):
    # jax.nn.gelu default (tanh approximation)
    return (0.5 * x * (1.0 + np.tanh(np.sqrt(2.0 / np.pi)
                                     * (x + 0.044715 * x ** 3)))).astype(np.float32)


def _ln(x, g, b, eps=1e-5):
    m = x.mean(-1, keepdims=True, dtype=np.float32)
    v = x.var(-1, keepdims=True, dtype=np.float32)
    return (x - m) / np.sqrt(v + eps) * g + b


def _bn(x, g, b, eps=1e-5):
    m = x.mean(0, dtype=np.float32)
    v = x.var(0, dtype=np.float32)
    return (x - m) / np.sqrt(v + eps) * g + b


class _Seg:
    """Presorted segment reducer: seg ids -> sorted perm + reduceat starts."""

    def __init__(self, seg, nseg):
        self.nseg = nseg
        self.perm = np.argsort(seg, kind="stable")
        ss = seg[self.perm]
        self.uniq, self.starts = np.unique(ss, return_index=True)

    def max(self, vals_sorted, fill):
        out = np.full((self.nseg,) + vals_sorted.shape[1:], fill, np.float32)
        out[self.uniq] = np.maximum.reduceat(vals_sorted, self.starts, axis=0)
        return out

    def sum(self, vals_sorted):
        out = np.zeros((self.nseg,) + vals_sorted.shape[1:], np.float32)
        out[self.uniq] = np.add.reduceat(vals_sorted, self.starts, axis=0)
        return out


def kernel(x0, x1, x2, y_base, W_in, b_in, ln_g, ln_b, W_kqv, b_kqv, W_krel,
           W_vrel, p_rel, W_out, b_out, skip, W_jk, b_jk, W_gate, b_gate,
           W_y1, b_y1, W_y2, b_y2, Wg1, bg1, g1, beta1, Wg2, bg2, g2, beta2,
           Wg3, bg3, ei0, ei1, ei2, ei3, batch0, batch1, batch2):
    f32 = np.float32
    xs = [np.asarray(x, f32) for x in (x0, x1, x2)]
    eis = [np.asarray(e) for e in (ei0, ei1, ei2, ei3)]
    batches = [np.asarray(b) for b in (batch0, batch1, batch2)]
    W_in, b_in, ln_g, ln_b = (np.asarray(a, f32) for a in (W_in, b_in, ln_g, ln_b))
    W_kqv, b_kqv, W_krel, W_vrel = (np.asarray(a, f32)
                                    for a in (W_kqv, b_kqv, W_krel, W_vrel))
    p_rel, W_out, b_out, skip = (np.asarray(a, f32)
                                 for a in (p_rel, W_out, b_out, skip))
    W_jk, b_jk, W_gate, b_gate = (np.asarray(a, f32)
                                  for a in (W_jk, b_jk, W_gate, b_gate))

    offs = [0, NS[0], NS[0] + NS[1]]
    total = sum(NS)

    # static edge structure: concat-order seg ids, presorted once
    segs_cat = np.concatenate(
        [eis[e][1] + offs[d_t] for e, (s_t, d_t) in enumerate(ET)])
    seg_red = _Seg(segs_cat, total)
    perm = seg_red.perm
    seg_sorted = segs_cat[perm]

    # per-type edge lists grouped by source type (for the fused projection)
    src_etypes = [[e for e, (s_t, _d) in enumerate(ET) if s_t == i]
                  for i in range(3)]            # [[0, 2], [1], [3]]
    # type 0 feeds two edge types: cheaper to ship raw q/k/v (768 cols) and
    # project k/v per edge type on host than to ship 4 folded blocks (1280)
    FUSED_MS = (3 * F, 3 * F, 3 * F)

    # proj_in
    xs = _dev_call(CIN, (F, F, F),
                   xs, np.concatenate([W_in[i] for i in range(3)], axis=1))
    xs = [xs[i] + b_in[i] for i in range(3)]
    layer_outs = [[] for _ in range(3)]

    for l in range(L):
        h = [_ln(xs[i], ln_g[l, i], ln_b[l, i]) for i in range(3)]
        # fold relation projections into the KQV weights, one call for all types
        wparts, bparts = [], []
        for i in range(3):
            Wk = W_kqv[l, i][:, :F]
            Wq = W_kqv[l, i][:, F:2 * F]
            Wv = W_kqv[l, i][:, 2 * F:]
            bk, bq, bv = b_kqv[l, i][:F], b_kqv[l, i][F:2 * F], b_kqv[l, i][2 * F:]
            if len(src_etypes[i]) > 1:       # raw q|k|v; host projects k/v
                cols, bs = [Wq, Wk, Wv], [bq, bk, bv]
            else:                            # single edge type: fold on device
                cols, bs = [Wq], [bq]
                for e in src_etypes[i]:
                    cols += [Wk @ W_krel[l, e], Wv @ W_vrel[l, e]]
                    bs += [bk @ W_krel[l, e], bv @ W_vrel[l, e]]
            wparts.append(np.concatenate(cols, axis=1))
            bparts.append(np.concatenate(bs))
        fused = _dev_call(F, FUSED_MS, h, np.concatenate(wparts, axis=1),
                          M8=2 * F, drk=True)
        q, vr = [], {}
        for i in range(3):
            yi = fused[i] + bparts[i]
            q.append(yi[:, :F].reshape(-1, H, DH))
            if len(src_etypes[i]) > 1:
                ki, vi = yi[:, F:2 * F], yi[:, 2 * F:3 * F]
                for e in src_etypes[i]:
                    vr[e] = ((ki @ W_krel[l, e]).reshape(-1, H, DH),
                             (vi @ W_vrel[l, e]).reshape(-1, H, DH))
            else:
                for j, e in enumerate(src_etypes[i]):
                    kr_e = yi[:, F + 2 * F * j:F + 2 * F * j + F]
                    vr_e = yi[:, 2 * F + 2 * F * j:2 * F + 2 * F * j + F]
                    vr[e] = (kr_e.reshape(-1, H, DH),
                             vr_e.reshape(-1, H, DH))
        alphas, vjs = [], []
        for e, (s_t, d_t) in enumerate(ET):
            src, dst = eis[e][0], eis[e][1]
            kr_e, vr_e = vr[e]
            a = ((q[d_t][dst] * kr_e[src]).sum(-1)
                 * p_rel[l, e] / np.sqrt(f32(DH))).astype(f32)
            alphas.append(a)
            vjs.append(vr_e[src])
        a = np.concatenate(alphas, 0)[perm]          # [E, H] dst-sorted
        vj = np.concatenate(vjs, 0)[perm]            # [E, H, DH]
        amax = seg_red.max(a, -np.inf)
        ex = np.exp(a - amax[seg_sorted])
        z = seg_red.sum(ex)
        attn = ex / (z[seg_sorted] + 1e-16)
        aggr = seg_red.sum((vj * attn[:, :, None]).reshape(-1, F))
        ga = [
            _gelu(aggr[offs[i]:offs[i] + NS[i]]) for i in range(3)]
        oi_p = _dev_call(F, (F, F, F),
                         ga, np.concatenate([W_out[l, i] for i in range(3)],
                                            axis=1),
                         M8=F, in8=True)
        new = []
        for i in range(3):
            al = 1.0 / (1.0 + np.exp(-skip[l, i]))
            oi = (al * (oi_p[i] + b_out[l, i]) + (1.0 - al) * h[i]).astype(f32)
            new.append(oi)
            layer_outs[i].append(oi)
        xs = new

    # JK + SAG pooling, algebraically folded (no device matmul needed):
    #   xs_f = cat @ W_jk + b_jk ; s = xs_f @ W_gate + b_gate
    #     == cat @ (W_jk @ W_gate) + (b_jk @ W_gate + b_gate)
    #   pooled = segsum(w * xs_f) = segsum(w * cat) @ W_jk + segsum(w) * b_jk
    pooled = []
    for i in range(3):
        cat = np.concatenate(layer_outs[i], axis=1)          # [N, L*F]
        wg_eff = W_jk[i] @ W_gate[i]                          # [L*F]
        s = cat @ wg_eff + (b_jk[i] @ W_gate[i] + b_gate[i])  # [N]
        sr = _Seg(batches[i], B)
        ss = s[sr.perm]
        smax = sr.max(ss, -np.inf)
        ex = np.exp(ss - smax[batches[i][sr.perm]])
        z = sr.sum(ex)
        w = ex / (z[batches[i][sr.perm]] + 1e-16)
        wc = sr.sum(w[:, None] * cat[sr.perm])                # [B, L*F]
        wsum = sr.sum(w[:, None])                             # [B, 1]
        pooled.append(wc @ W_jk[i] + wsum * b_jk[i])

    hy = np.asarray(y_base, f32) @ np.asarray(W_y1, f32) + np.asarray(b_y1, f32)
    hy = np.where(hy > 0, hy, 0.2 * hy)
    hy = hy @ np.asarray(W_y2, f32) + np.asarray(b_y2, f32)
    out = np.concatenate(pooled + [hy], axis=1).astype(f32)
    out = _gelu(_bn(out @ np.asarray(Wg1, f32) + np.asarray(bg1, f32),
                    np.asarray(g1, f32), np.asarray(beta1, f32)))
    out = _gelu(_bn(out @ np.asarray(Wg2, f32) + np.asarray(bg2, f32),
                    np.asarray(g2, f32), np.asarray(beta2, f32)))
    return (out @ np.asarray(Wg3, f32) + np.asarray(bg3, f32)).squeeze(1)



# revision 43
# speedup vs baseline: 1.3113x; 1.3113x over previous
"""HGT GNN kernel for 8 Trainium2 NeuronCores — v2.

Device does all heavy dense matmuls in bf16 via three cached Bass/Tile
programs, each covering all three node types in one call (per-row-block
weight selection, exact per-core row counts, no padding):

  pin   : x[Ni,128]    @ W_in[i]                      -> 256 cols out
  fused : h[Ni,256]    @ [Wq | Wk@Wkrel_e | Wv@Wvrel_e] -> q/kr/vr in one shot
          (the K/V relation projections are folded into the KQV weights,
          removing the separate k/v matmuls and 8 relation matmuls/layer)
  wout  : gelu(aggr)   @ W_out[l,i]                   -> 256 cols out

JumpingKnowledge + SAG pooling are algebraically folded to the host side:
gate scores use W_jk@W_gate, and pooled = segsum(w*cat) @ W_jk, so the
[170000,1024]@[1024,256] JK matmul disappears entirely.

Irregular glue (edge gather / segment softmax / scatter) and the tiny
BatchNorm head run on host in fp32, with edges presorted by destination.
"""

import contextlib
import ctypes
import sys
import types

import numpy as np
import ml_dtypes

import concourse.bass as bass
import concourse.mybir as mybir
import concourse.tile as tile
from concourse.bass_utils import run_bass_kernel_spmd
from concourse.vector_clock import ScopedClock

BF16 = ml_dtypes.bfloat16


# ------------------------------------------------------- ntff profile shim
def _install_ntff_shim():
    """This image's antenv lacks axon_hooks; recreate the NTFF profile hook
    via the libaxon_pjrt.so C ABI so trace=True yields exec_time_ns."""
    try:
        from antenv.axon_hooks import get_axon_ntff_profile_hook  # noqa: F401
        return
    except ImportError:
        pass

    so_path = "/opt/axon/libaxon_pjrt.so"
    try:
        lib = ctypes.CDLL(so_path)
    except OSError:
        return
    if not hasattr(lib, "axon_start_nrt_profile"):
        return
    lib.axon_start_nrt_profile.argtypes = [ctypes.POINTER(ctypes.c_int64),
                                           ctypes.c_size_t]
    lib.axon_start_nrt_profile.restype = ctypes.c_int64
    lib.axon_stop_nrt_profile.argtypes = [ctypes.c_char_p]
    lib.axon_stop_nrt_profile.restype = ctypes.c_int64

    @contextlib.contextmanager
    def _hook(output_dir, device_ids):
        import jax
        jax.devices()
        if device_ids:
            ids = (ctypes.c_int64 * len(device_ids))(*device_ids)
            rc = lib.axon_start_nrt_profile(ids, len(device_ids))
        else:
            rc = lib.axon_start_nrt_profile(None, 0)
        if rc != 0:
            raise RuntimeError(f"axon_start_nrt_profile rc={rc}")
        try:
            yield
        finally:
            n = lib.axon_stop_nrt_profile(str(output_dir).encode())
            if n <= 0:
                print(f"ntff profile capture wrote {n} files", file=sys.stderr)

    mod = types.ModuleType("antenv.axon_hooks")
    mod.get_axon_ntff_profile_hook = lambda: _hook
    mod.set_axon_ntff_profile_hook = lambda h: None
    sys.modules["antenv.axon_hooks"] = mod
    import antenv
    antenv.axon_hooks = mod

    import concourse.bass_utils as bu
    bu.upload_artifacts = lambda tmpdir: tmpdir


_install_ntff_shim()

# model dims (hardcoded per contract)
H, DH, F, L, B = 4, 64, 256, 4, 64
NS = [80000, 60000, 30000]
ET = [(0, 1), (1, 0), (0, 2), (2, 0)]
NE = [320000, 320000, 160000, 160000]
CIN = 128

N_CORES = 8
PC = [n // N_CORES for n in NS]          # 10000, 7500, 3750 rows/core
RTOT = sum(PC)                           # 21250
COFF = [0, PC[0], PC[0] + PC[1]]         # per-type col offsets in device layout
FB = 512                                 # free-dim block = one PSUM bank exactly


# ---------------------------------------------------------------- tile drain fix
def _install_tilefix():
    """This container's walrus rejects >1 sync wait on TPB_CTRL-class
    instructions; spread the Tile tail-drain waits across SP nops."""

    def _drain_and_barrier_split(self, tick_clock, wait_clock):
        """Minimal teardown: drain the two DMA-issuing engines (guarantees
        all output DMAs complete), one engine barrier, then clear sems from
        gpsimd without a second barrier or per-tile-clock waits."""
        nc = self.nc
        nc.sync.drain()
        nc.gpsimd.drain()
        nc.all_engine_barrier()
        assert self.sems is not None
        popped = nc._tile_sem_poison_stack.pop()
        assert popped is self._sem_poison
        sems = list(self.sems.allocated().values())
        sem_nums = [s.num if hasattr(s, "num") else s for s in sems]
        from concourse.bass import compact_to_ranges
        for sem_range in compact_to_ranges(sem_nums):
            nc.gpsimd.dma_reset(sem_range)
            nc.gpsimd.sem_clear(sem_range)
        nc._state.prepend_free_semaphores(sem_nums)

    tile.TileContext._drain_and_barrier = _drain_and_barrier_split


_install_tilefix()


def _split_multiwaits(nc):
    """Walrus here allows only one sync wait per instruction: move extra
    waits onto same-engine nops placed immediately before the instruction."""
    for f in nc.m.functions:
        for bb in f.blocks:
            insts = list(bb.instructions)
            out = []
            for inst in insts:
                si = getattr(inst, "sync_info", None)
                if si and si.on_wait and len(si.on_wait) > 1:
                    extra, keep = si.on_wait[:-1], si.on_wait[-1:]
                    si.on_wait = keep
                    for w in extra:
                        nop = nc.engines[inst.engine].nop(nofuse=True)
                        cur = nc.cur_bb.bb.instructions
                        assert cur[-1] is nop.ins
                        cur.pop()
                        nop.ins.sync_info = type(si)(on_wait=[w], on_update=[])
                        out.append(nop.ins)
                out.append(inst)
            bb.instructions[:] = out


# ---------------------------------------------------------------- device matmul
_PROGS = {}
_CALL_COUNTS = {}


GW = 2048  # column group width: 4 PSUM banks per (group, mc); one in/out DMA per group
OUT_ENG = "gpsimd"  # which engine issues output DMAs: sync | scalar | gpsimd


def _groups():
    """(type, group_col0, group_width) covering each type's per-core cols."""
    out = []
    for t in range(3):
        n, c0 = PC[t], COFF[t]
        g = 0
        while g < n:
            w = min(GW, n - g)
            out.append((t, c0 + g, w))
            g += w
    return out


WSCALE = 16.0  # fp8 weight pre-scale for DoubleRow k-columns


def _build_multi(K, Ms, M8, in8=False, drk=False):
    """One SPMD program: per-type matmuls over the concatenated per-core
    rows. xt [K, RTOT] bf16 (feature-major), w [K, sum(Ms)] bf16.
    The first M8 output rows of each type are written to yt8 (fp8e4),
    the rest to yt (bf16). DMA is coalesced at GW-column granularity
    (MB-scale transfers); each (group, mc) accumulates into a 2-bank PSUM
    tile drained by a single wide PSUM->SBUF cast, alternating DVE/ACT.
    ~7us of scratch warmup matmuls run during the input DMA ramp so the
    PE HAM clock-gate is at 8/8 before the first real matmul."""
    dt = mybir.dt.bfloat16
    dt8 = mybir.dt.float8e4
    KC = K // 128
    Mtot, Mmax = sum(Ms), max(Ms)
    MCmax = Mmax // 128
    MC8 = M8 // 128                      # leading fp8 out blocks per type
    MC16max = MCmax - MC8
    woff = [0, Ms[0], Ms[0] + Ms[1]]
    dtin = dt8 if in8 else dt
    # drk: mc blocks [MC8//2, MC8) of each type (the fused "k" columns) are
    # computed as a single DoubleRow fp8 matmul over both K halves, using a
    # device-side fp8 cast of x and pre-scaled fp8 weights from `w8`.
    DRLO = MC8 // 2 if drk else None
    nc = bass.Bass("TRN2", target_bir_lowering=False, debug=False,
                   num_devices=N_CORES)
    # group-blocked contiguous layouts: element (p, kc|mc, c) of column-group
    # g lives at flat offset g0*K + (p*KC + kc)*gwid + c, so every DMA is one
    # contiguous DRAM segment per partition (max-size descriptors).
    xt = nc.dram_tensor("xt", [RTOT * K], dtin, kind="ExternalInput")
    w = nc.dram_tensor("w", [K, Mtot], dt, kind="ExternalInput")
    if drk:
        MDR = 128 * (MC8 - DRLO) * len(Ms)    # DR cols across all types
        w8 = nc.dram_tensor("w8", [K, MDR], dt8, kind="ExternalInput")
    if MC8 > 0:
        yt8 = nc.dram_tensor("yt8", [RTOT * M8], dt8, kind="ExternalOutput")
    if MC16max > 0:
        M16u = Mmax - M8
        yt = nc.dram_tensor("yt", [RTOT * M16u], dt, kind="ExternalOutput")
    grs = _groups()

    def xblk(g0, gwid):
        return xt[g0 * K:(g0 + gwid) * K].rearrange(
            "(p kc c) -> p kc c", p=128, kc=KC)

    def y8blk(g0, gwid):
        return yt8[g0 * M8:(g0 + gwid) * M8].rearrange(
            "(p mc c) -> p mc c", p=128, mc=MC8)

    def y16blk(g0, gwid):
        return yt[g0 * M16u:(g0 + gwid) * M16u].rearrange(
            "(p mc c) -> p mc c", p=128, mc=MC16max)
    # og tile is MCmax*GW*2 bytes/partition; budget ~208KB/partition
    op_bufs = 4 if MCmax >= 10 else (6 if MCmax >= 6 else 8)
    with tile.TileContext(nc) as tc:
        with (
            tc.tile_pool(name="wp", bufs=1) as wp,
            tc.tile_pool(name="xp", bufs=6) as xp,
            tc.tile_pool(name="op", bufs=op_bufs) as op,
            tc.tile_pool(name="ps", bufs=4, space="PSUM") as ps,
        ):
            # --- PE warmup: scratch matmuls with no DMA dependencies ---
            # (shares the "ph" PSUM ring so it costs no extra PSUM space)
            # ~12 N=256 matmuls span one HAM window at the cold rate, so the
            # clock-gate is at 8/8 just as the first real inputs land.
            wu = wp.tile([128, 384], dt)
            nc.vector.memset(wu[:, :], 0.0)
            wups = ps.tile([128, 2 * FB], mybir.dt.float32, space="PSUM",
                           name="ph")
            for _ in range(13):
                nc.tensor.matmul(out=wups[:, :256], lhsT=wu[:, :128],
                                 rhs=wu[:, 128:384],
                                 start=True, stop=True)
            wt = wp.tile([128, KC * Mtot], dt)
            M0 = Ms[0]
            wdmas = []
            for kc in range(KC):   # first-needed weight cols (type 0)
                wdmas.append((wt[:, kc * Mtot:kc * Mtot + M0],
                              w[kc * 128:(kc + 1) * 128, :M0]))
            if drk:
                assert KC == 2, "DoubleRow path assumes K=256"
                wt8 = wp.tile([128, KC, MDR], dt8)
                for kc in range(KC):
                    wdmas.append((wt8[:, kc, :],
                                  w8[kc * 128:(kc + 1) * 128, :]))
            for kc in range(KC):   # remaining weight cols
                if Mtot > M0:
                    wdmas.append((wt[:, kc * Mtot + M0:(kc + 1) * Mtot],
                                  w[kc * 128:(kc + 1) * 128, M0:]))
            HW2 = 2 * FB          # half-group: one 2-bank PSUM tile
            xgs = {}
            for gi, (t, g0, gwid) in enumerate(grs):
                MC = Ms[t] // 128
                MC16 = MC - MC8
                xg = xp.tile([128, KC, GW], dtin, name="xg")
                xgs[gi] = xg
                xb = xblk(g0, gwid)
                if gi == 0:
                    # first matmul's weights first, then a fine-grained
                    # first x chunk so matmul 0 starts early
                    for o, i_ in wdmas[:KC]:
                        nc.sync.dma_start(out=o, in_=i_)
                    h1 = min(FB, gwid)
                    nc.sync.dma_start(out=xg[:, :, :h1], in_=xb[:, :, :h1])
                    for o, i_ in wdmas[KC:]:
                        nc.sync.dma_start(out=o, in_=i_)
                    if gwid > h1:
                        nc.sync.dma_start(out=xg[:, :, h1:gwid],
                                          in_=xb[:, :, h1:gwid])
                else:
                    nc.sync.dma_start(out=xg[:, :, :gwid],
                                      in_=xb[:, :, :gwid])
                if drk:  # fp8 copy of x for the DoubleRow k-matmuls
                    xg8 = xp.tile([128, KC, GW], dt8, name="xg8", bufs=4)
                    nc.gpsimd.tensor_copy(out=xg8[:, :, :gwid],
                                          in_=xg[:, :, :gwid])
                og8 = (op.tile([128, MC8, GW], dt8, name="og8")
                       if MC8 else None)
                og16 = (op.tile([128, MC16, GW], dt, name="og16")
                        if MC16 else None)
                last = gi == len(grs) - 1
                for mc in range(MC):
                    if mc < MC8:
                        og, ogmc = og8, mc
                    else:
                        og, ogmc = og16, mc - MC8
                    isdr = drk and DRLO <= mc < MC8
                    for hb, h0 in enumerate(range(0, gwid, HW2)):
                        hw = min(HW2, gwid - h0)
                        ph = ps.tile([128, HW2], mybir.dt.float32,
                                     space="PSUM")
                        if isdr:   # one DoubleRow matmul covers both K halves
                            mdr = (t * (MC8 - DRLO) + (mc - DRLO)) * 128
                            for boff in range(h0, h0 + hw, FB):
                                fb = min(FB, h0 + hw - boff)
                                nc.tensor.matmul(
                                    out=ph[:, boff - h0:boff - h0 + fb],
                                    lhsT=wt8[:, :, mdr:mdr + 128],
                                    rhs=xg8[:, :, boff:boff + fb],
                                    start=True, stop=True,
                                    perf_mode=mybir.MatmulPerfMode.DoubleRow)
                        else:
                            for kc in range(KC):   # blocks inner: LDW shared
                                wcol = kc * Mtot + woff[t] + mc * 128
                                for boff in range(h0, h0 + hw, FB):
                                    fb = min(FB, h0 + hw - boff)
                                    nc.tensor.matmul(
                                        out=ph[:, boff - h0:boff - h0 + fb],
                                        lhsT=wt[:, wcol:wcol + 128],
                                        rhs=xg[:, kc, boff:boff + fb],
                                        start=(kc == 0), stop=(kc == KC - 1))
                        # the two halves of each mc drain on different engines
                        # (DoubleRow columns also undo the weight pre-scale)
                        if hb % 2 == 0:
                            if isdr:
                                nc.vector.tensor_scalar_mul(
                                    out=og[:, ogmc, h0:h0 + hw],
                                    in0=ph[:, :hw], scalar1=1.0 / WSCALE)
                            else:
                                nc.vector.tensor_copy(
                                    out=og[:, ogmc, h0:h0 + hw],
                                    in_=ph[:, :hw])
                        else:
                            if isdr:
                                nc.scalar.mul(out=og[:, ogmc, h0:h0 + hw],
                                              in_=ph[:, :hw],
                                              mul=1.0 / WSCALE)
                            else:
                                nc.scalar.copy(out=og[:, ogmc, h0:h0 + hw],
                                               in_=ph[:, :hw])
                    if last:  # drain the final group per-mc: shorter tail
                        # alternate issuing engines so descriptor generation
                        # for the final DMAs overlaps
                        eng = ("gpsimd", "sync")[mc % 2]
                        if mc < MC8:
                            getattr(nc, eng).dma_start(
                                out=y8blk(g0, gwid)[:, mc:mc + 1, :gwid],
                                in_=og8[:, mc:mc + 1, :gwid])
                        else:
                            getattr(nc, eng).dma_start(
                                out=y16blk(g0, gwid)[:, mc - MC8:mc - MC8 + 1,
                                                     :gwid],
                                in_=og16[:, mc - MC8:mc - MC8 + 1, :gwid])
                if not last:
                    if MC8:
                        getattr(nc, OUT_ENG).dma_start(
                            out=y8blk(g0, gwid)[:, :MC8, :gwid],
                            in_=og8[:, :MC8, :gwid])
                    if MC16:
                        getattr(nc, OUT_ENG).dma_start(
                            out=y16blk(g0, gwid)[:, :MC16, :gwid],
                            in_=og16[:, :MC16, :gwid])
    _split_multiwaits(nc)
    return nc


def _make_runner(nc, Mmax, M8, in8=False, drk=False):
    """Persistent jitted SPMD executor for one program (built once;
    per-call dispatch is then cheap)."""
    import jax
    import jax.numpy as jnp
    from jax.experimental.shard_map import shard_map
    from jax.sharding import Mesh, PartitionSpec
    from concourse.bass2jax import (_bass_exec_p, partition_id_tensor,
                                    install_neuronx_cc_hook)

    install_neuronx_cc_hook()
    F8 = ml_dtypes.float8_e4m3fn
    M16 = Mmax - M8
    out_names, out_avals, zdts, zrows = [], [], [], []
    if M8:
        out_names.append("yt8")
        out_avals.append(jax.core.ShapedArray((RTOT * M8,), F8))
        zdts.append(F8)
        zrows.append(M8)
    if M16:
        out_names.append("yt")
        out_avals.append(jax.core.ShapedArray((RTOT * M16,), BF16))
        zdts.append(BF16)
        zrows.append(M16)
    pname = nc.partition_id_tensor.name if nc.partition_id_tensor else None
    wnames = ["w", "w8"] if drk else ["w"]
    in_names = ["xt"] + wnames + out_names + ([pname] if pname else [])
    nin = 1 + len(wnames)

    def _body(xt, *rest):
        operands = [xt, *rest]
        if pname is not None:
            operands.append(partition_id_tensor())
        outs = _bass_exec_p.bind(
            *operands, out_avals=tuple(out_avals), in_names=tuple(in_names),
            out_names=tuple(out_names), lowering_input_output_aliases=(),
            sim_require_finite=False, sim_require_nnan=False, nc=nc)
        return tuple(outs)

    devices = jax.devices()[:N_CORES]
    mesh = Mesh(np.asarray(devices), ("core",))
    nouts = len(out_names)
    sharded = jax.jit(
        shard_map(_body, mesh=mesh,
                  in_specs=(PartitionSpec("core"),) * (nin + nouts),
                  out_specs=(PartitionSpec("core"),) * nouts,
                  check_rep=False),
        keep_unused=True)
    yzs = [jax.device_put(
        np.zeros(N_CORES * RTOT * r, zdt),
        jax.sharding.NamedSharding(mesh, PartitionSpec("core")))
        for r, zdt in zip(zrows, zdts)]

    def run(xt_all, wcat, w8cat=None):
        # xt_all [N_CORES * RTOT*K] packed; wcat [K, Mtot] bf16 (replicated)
        def rep(a):
            return np.ascontiguousarray(
                np.broadcast_to(a, (N_CORES,) + a.shape)
            ).reshape((N_CORES * a.shape[0],) + a.shape[1:])
        ws = [rep(wcat)] + ([rep(w8cat)] if drk else [])
        outs = sharded(xt_all, *ws, *yzs)
        r8 = np.asarray(outs[0]) if M8 else None
        r16 = np.asarray(outs[-1]) if M16 else None
        return r8, r16

    return run


def _get_prog(K, Ms, M8=0, in8=False, drk=False):
    key = (K,) + tuple(Ms) + (M8, in8, drk)
    if key not in _PROGS:
        nc = _build_multi(K, Ms, M8, in8, drk)
        _PROGS[key] = (nc, _make_runner(nc, max(Ms), M8, in8, drk),
                       Ms, M8, in8, drk)
    return _PROGS[key]


def _dev_call(K, Ms, xs_by_type, wcat, M8=0, in8=False, drk=False):
    """xs_by_type: list of 3 host arrays [N_i, K] f32.  wcat [K, sum(Ms)] f32.
    Returns list of 3 arrays [N_i, Ms[i]] f32; the first M8 output cols of
    each type round-trip through fp8e4m3; in8 ships x as fp8e4m3; drk runs
    the k-columns (second quarter of M8) as DoubleRow fp8 matmuls."""
    key = (K,) + tuple(Ms) + (M8, in8, drk)
    _, run, _, _, _, _ = _get_prog(K, Ms, M8, in8, drk)
    _CALL_COUNTS[key] = _CALL_COUNTS.get(key, 0) + 1
    Mmax = max(Ms)
    M16 = Mmax - M8
    KC = K // 128
    MC8 = M8 // 128
    F8 = ml_dtypes.float8_e4m3fn
    dtin = F8 if in8 else BF16
    grs = _groups()
    # pack: per core, per column-group, block layout (p, kc, c) contiguous
    xt_all = np.empty((N_CORES, RTOT * K), dtin)
    for c in range(N_CORES):
        for t in range(3):
            pc = PC[t]
            xkm = np.ascontiguousarray(
                xs_by_type[t][c * pc:(c + 1) * pc].T).astype(dtin)  # [K, pc]
            xkm = xkm.reshape(KC, 128, pc).transpose(1, 0, 2)       # [p,kc,pc]
            for (gt, g0, gwid) in grs:
                if gt != t:
                    continue
                lg = g0 - COFF[t]
                xt_all[c, g0 * K:(g0 + gwid) * K] = \
                    xkm[:, :, lg:lg + gwid].ravel()
    if drk:
        F2 = M8 // 2                       # k-cols are [F2, 2*F2) per type
        woffs = np.cumsum([0] + list(Ms))
        w8cat = np.concatenate(
            [wcat[:, woffs[t] + F2:woffs[t] + 2 * F2]
             for t in range(len(Ms))], axis=1)
        w8cat = (w8cat * WSCALE).astype(F8)
        y8all, y16all = run(xt_all.reshape(-1), wcat.astype(BF16), w8cat)
    else:
        y8all, y16all = run(xt_all.reshape(-1), wcat.astype(BF16))
    outs = []
    for t in range(3):
        pc, Mt = PC[t], Ms[t]
        parts = []
        for c in range(N_CORES):
            arr = np.empty((pc, Mt), np.float32)
            for (gt, g0, gwid) in grs:
                if gt != t:
                    continue
                lg = g0 - COFF[t]
                if M8:
                    blk = y8all[c * RTOT * M8 + g0 * M8:
                                c * RTOT * M8 + (g0 + gwid) * M8]
                    blk = blk.reshape(128, MC8, gwid).transpose(2, 1, 0)
                    arr[lg:lg + gwid, :M8] = \
                        blk.reshape(gwid, M8).astype(np.float32)
                if Mt - M8 > 0:
                    blk = y16all[c * RTOT * M16 + g0 * M16:
                                 c * RTOT * M16 + (g0 + gwid) * M16]
                    blk = blk.reshape(128, M16 // 128, gwid).transpose(2, 1, 0)
                    arr[lg:lg + gwid, M8:M8 + M16] = \
                        blk.reshape(gwid, M16).astype(np.float32)
            parts.append(arr)
        outs.append(np.concatenate(parts, axis=0))
    return outs


def _timed_mm_ns():
    """Three traced runs per cached program (min, to reject power-state
    outliers); returns sum(count * exec_ns)."""
    total = 0
    for key, (nc, _run, Ms, M8, in8, drk) in _PROGS.items():
        K = key[0]
        dtin = ml_dtypes.float8_e4m3fn if in8 else BF16
        in_maps = [{"xt": np.zeros(RTOT * K, dtin),
                    "w": np.zeros((K, sum(Ms)), BF16)}
                   for _ in range(N_CORES)]
        if drk:
            F8z = np.zeros((K, (M8 // 2) * len(Ms)),
                           ml_dtypes.float8_e4m3fn)
            for m in in_maps:
                m["w8"] = F8z
        times = []
        for _ in range(3):
            r = run_bass_kernel_spmd(nc, in_maps, list(range(N_CORES)),
                                     trace=True)
            if r.exec_time_ns:
                times.append(r.exec_time_ns)
        if times:
            total += min(times) * _CALL_COUNTS.get(key, 0)
    return total


# ---------------------------------------------------------------- host helpers
def _gelu(x):
    # jax.nn.gelu default (tanh approximation)
    return (0.5 * x * (1.0 + np.tanh(np.sqrt(2.0 / np.pi)
                                     * (x + 0.044715 * x ** 3)))).astype(np.float32)


def _ln(x, g, b, eps=1e-5):
    m = x.mean(-1, keepdims=True, dtype=np.float32)
    v = x.var(-1, keepdims=True, dtype=np.float32)
    return (x - m) / np.sqrt(v + eps) * g + b


def _bn(x, g, b, eps=1e-5):
    m = x.mean(0, dtype=np.float32)
    v = x.var(0, dtype=np.float32)
    return (x - m) / np.sqrt(v + eps) * g + b


class _Seg:
    """Presorted segment reducer: seg ids -> sorted perm + reduceat starts."""

    def __init__(self, seg, nseg):
        self.nseg = nseg
        self.perm = np.argsort(seg, kind="stable")
        ss = seg[self.perm]
        self.uniq, self.starts = np.unique(ss, return_index=True)

    def max(self, vals_sorted, fill):
        out = np.full((self.nseg,) + vals_sorted.shape[1:], fill, np.float32)
        out[self.uniq] = np.maximum.reduceat(vals_sorted, self.starts, axis=0)
        return out

    def sum(self, vals_sorted):
        out = np.zeros((self.nseg,) + vals_sorted.shape[1:], np.float32)
        out[self.uniq] = np.add.reduceat(vals_sorted, self.starts, axis=0)
        return out


def kernel(x0, x1, x2, y_base, W_in, b_in, ln_g, ln_b, W_kqv, b_kqv, W_krel,
           W_vrel, p_rel, W_out, b_out, skip, W_jk, b_jk, W_gate, b_gate,
           W_y1, b_y1, W_y2, b_y2, Wg1, bg1, g1, beta1, Wg2, bg2, g2, beta2,
           Wg3, bg3, ei0, ei1, ei2, ei3, batch0, batch1, batch2):
    f32 = np.float32
    xs = [np.asarray(x, f32) for x in (x0, x1, x2)]
    eis = [np.asarray(e) for e in (ei0, ei1, ei2, ei3)]
    batches = [np.asarray(b) for b in (batch0, batch1, batch2)]
    W_in, b_in, ln_g, ln_b = (np.asarray(a, f32) for a in (W_in, b_in, ln_g, ln_b))
    W_kqv, b_kqv, W_krel, W_vrel = (np.asarray(a, f32)
                                    for a in (W_kqv, b_kqv, W_krel, W_vrel))
    p_rel, W_out, b_out, skip = (np.asarray(a, f32)
                                 for a in (p_rel, W_out, b_out, skip))
    W_jk, b_jk, W_gate, b_gate = (np.asarray(a, f32)
                                  for a in (W_jk, b_jk, W_gate, b_gate))

    offs = [0, NS[0], NS[0] + NS[1]]
    total = sum(NS)

    # static edge structure: concat-order seg ids, presorted once
    segs_cat = np.concatenate(
        [eis[e][1] + offs[d_t] for e, (s_t, d_t) in enumerate(ET)])
    seg_red = _Seg(segs_cat, total)
    perm = seg_red.perm
    seg_sorted = segs_cat[perm]

    # per-type edge lists grouped by source type (for the fused projection)
    src_etypes = [[e for e, (s_t, _d) in enumerate(ET) if s_t == i]
                  for i in range(3)]            # [[0, 2], [1], [3]]
    # type 0 feeds two edge types: cheaper to ship raw q/k/v (768 cols) and
    # project k/v per edge type on host than to ship 4 folded blocks (1280)
    FUSED_MS = (3 * F, 3 * F, 3 * F)

    # proj_in
    xs = _dev_call(CIN, (F, F, F),
                   xs, np.concatenate([W_in[i] for i in range(3)], axis=1))
    xs = [xs[i] + b_in[i] for i in range(3)]
    layer_outs = [[] for _ in range(3)]

    for l in range(L):
        h = [_ln(xs[i], ln_g[l, i], ln_b[l, i]) for i in range(3)]
        # fold relation projections into the KQV weights, one call for all types
        wparts, bparts = [], []
        for i in range(3):
            Wk = W_kqv[l, i][:, :F]
            Wq = W_kqv[l, i][:, F:2 * F]
            Wv = W_kqv[l, i][:, 2 * F:]
            bk, bq, bv = b_kqv[l, i][:F], b_kqv[l, i][F:2 * F], b_kqv[l, i][2 * F:]
            if len(src_etypes[i]) > 1:       # raw q|k|v; host projects k/v
                cols, bs = [Wq, Wk, Wv], [bq, bk, bv]
            else:                            # single edge type: fold on device
                cols, bs = [Wq], [bq]
                for e in src_etypes[i]:
                    cols += [Wk @ W_krel[l, e], Wv @ W_vrel[l, e]]
                    bs += [bk @ W_krel[l, e], bv @ W_vrel[l, e]]
            wparts.append(np.concatenate(cols, axis=1))
            bparts.append(np.concatenate(bs))
        fused = _dev_call(F, FUSED_MS, h, np.concatenate(wparts, axis=1),
                          M8=2 * F)
        q, vr = [], {}
        for i in range(3):
            yi = fused[i] + bparts[i]
            q.append(yi[:, :F].reshape(-1, H, DH))
            if len(src_etypes[i]) > 1:
                ki, vi = yi[:, F:2 * F], yi[:, 2 * F:3 * F]
                for e in src_etypes[i]:
                    vr[e] = ((ki @ W_krel[l, e]).reshape(-1, H, DH),
                             (vi @ W_vrel[l, e]).reshape(-1, H, DH))
            else:
                for j, e in enumerate(src_etypes[i]):
                    kr_e = yi[:, F + 2 * F * j:F + 2 * F * j + F]
                    vr_e = yi[:, 2 * F + 2 * F * j:2 * F + 2 * F * j + F]
                    vr[e] = (kr_e.reshape(-1, H, DH),
                             vr_e.reshape(-1, H, DH))
        alphas, vjs = [], []
        for e, (s_t, d_t) in enumerate(ET):
            src, dst = eis[e][0], eis[e][1]
            kr_e, vr_e = vr[e]
            a = ((q[d_t][dst] * kr_e[src]).sum(-1)
                 * p_rel[l, e] / np.sqrt(f32(DH))).astype(f32)
            alphas.append(a)
            vjs.append(vr_e[src])
        a = np.concatenate(alphas, 0)[perm]          # [E, H] dst-sorted
        vj = np.concatenate(vjs, 0)[perm]            # [E, H, DH]
        amax = seg_red.max(a, -np.inf)
        ex = np.exp(a - amax[seg_sorted])
        z = seg_red.sum(ex)
        attn = ex / (z[seg_sorted] + 1e-16)
        aggr = seg_red.sum((vj * attn[:, :, None]).reshape(-1, F))
        ga = [
            _gelu(aggr[offs[i]:offs[i] + NS[i]]) for i in range(3)]
        oi_p = _dev_call(F, (F, F, F),
                         ga, np.concatenate([W_out[l, i] for i in range(3)],
                                            axis=1),
                         M8=F, in8=True)
        new = []
        for i in range(3):
            al = 1.0 / (1.0 + np.exp(-skip[l, i]))
            oi = (al * (oi_p[i] + b_out[l, i]) + (1.0 - al) * h[i]).astype(f32)
            new.append(oi)
            layer_outs[i].append(oi)
        xs = new

    # JK + SAG pooling, algebraically folded (no device matmul needed):
    #   xs_f = cat @ W_jk + b_jk ; s = xs_f @ W_gate + b_gate
    #     == cat @ (W_jk @ W_gate) + (b_jk @ W_gate + b_gate)
    #   pooled = segsum(w * xs_f) = segsum(w * cat) @ W_jk + segsum(w) * b_jk
    pooled = []
    for i in range(3):
        cat = np.concatenate(layer_outs[i], axis=1)          # [N, L*F]
        wg_eff = W_jk[i] @ W_gate[i]                          # [L*F]
        s = cat @ wg_eff + (b_jk[i] @ W_gate[i] + b_gate[i])  # [N]
        sr = _Seg(batches[i], B)
        ss = s[sr.perm]
        smax = sr.max(ss, -np.inf)
        ex = np.exp(ss - smax[batches[i][sr.perm]])
        z = sr.sum(ex)
        w = ex / (z[batches[i][sr.perm]] + 1e-16)
        wc = sr.sum(w[:, None] * cat[sr.perm])                # [B, L*F]
        wsum = sr.sum(w[:, None])                             # [B, 1]
        pooled.append(wc @ W_jk[i] + wsum * b_jk[i])

    hy = np.asarray(y_base, f32) @ np.asarray(W_y1, f32) + np.asarray(b_y1, f32)
    hy = np.where(hy > 0, hy, 0.2 * hy)
    hy = hy @ np.asarray(W_y2, f32) + np.asarray(b_y2, f32)
    out = np.concatenate(pooled + [hy], axis=1).astype(f32)
    out = _gelu(_bn(out @ np.asarray(Wg1, f32) + np.asarray(bg1, f32),
                    np.asarray(g1, f32), np.asarray(beta1, f32)))
    out = _gelu(_bn(out @ np.asarray(Wg2, f32) + np.asarray(bg2, f32),
                    np.asarray(g2, f32), np.asarray(beta2, f32)))
    return (out @ np.asarray(Wg3, f32) + np.asarray(bg3, f32)).squeeze(1)

